# revision 39
# baseline (speedup 1.0000x reference)
"""LogicLayer Trainium2 kernel (v4: pair-gather + form-B, no fused STT).

out[b, n] = A0 + A1*a1 + A2*a2 + Ap*a1*a2  (A = softmax(w) @ C, host-precomputed)

Structure (per core: 1024 neurons x 2048 batch, 8 slots of 128 neurons):
  - Gathers via gpsimd.indirect_dma_start (~1.1 us issue per 128-row call).
    Descriptor count cut by PAIR-GATHERING: capacity-3 matching on the neuron
    graph (edges = (i1,i2)); matched operand pairs stored adjacent in a pair
    table so ONE descriptor fetches both operands: row = [f16 xF | u8 xU]
    (6KB). Slots 0-4 are paired (1 call each); slots 5-6 unpaired form-B
    (2 calls); slot 7 = form-A fallback for the worst-|A1*A2/Ap| neurons
    (factored form is numerically unsafe there). 11 calls/core.
  - form-B per slot (validated at rel_fro 2.3e-3):
      t2  = (253*Ap/255)*g2u8 + 253*A_F     ACT (u8 upconvert), slot6 on DVE
      t1  = g1f16 + A_U/Ap                  DVE tensor_scalar add (4x)
      q   = t1 * t2                         DVE tensor_tensor (2x), slot6 gpsimd
      u8  = q + (253*K + 1.5)               DVE tensor_scalar add, u8 out (2x)
  - form-A slot 7: u,v on ACT; m = u*g2 (TT); out = m + v written u8 directly
    (TT u8-out runs 2x, measured).
  - All outputs u8 = 253*out + 1.5 (HW rounds to nearest); host decodes.
  Measured op costs: ACT 2.08us; DVE TS f16 0.81, TS u8-out 1.28, TT 1.22;
  gpsimd TT ~4.4; NEVER u8-out on gpsimd (29us ucode path + DVE contention);
  scalar_tensor_tensor runs 1x (2.35us) - do not use.
"""

import numpy as np

BATCH = 2048
NIN = 8192
NNEUR = 8192
NCORES = 8
NN = NNEUR // NCORES
NB = BATCH
SLOTS = NN // 128
KG = 6                 # paired slots 0..5
U8S = 253.0
U8B = 1.5

GATE_COEF = np.array(
    [
        [0, 0, 0, 0], [0, 0, 0, 1], [0, 1, 0, -1], [0, 1, 0, 0],
        [0, 0, 1, -1], [0, 0, 1, 0], [0, 1, 1, -2], [0, 1, 1, -1],
        [1, -1, -1, 1], [1, -1, -1, 2], [1, 0, -1, 0], [1, 0, -1, 1],
        [1, -1, 0, 0], [1, -1, 0, 1], [1, 0, 0, -1], [1, 0, 0, 0],
    ],
    dtype=np.float64,
)

_CACHE = {}


def _build_nc(np_pairs):
    import concourse.bacc as bacc
    import concourse.bass as bass
    import concourse.mybir as mybir
    from concourse.tile import TileContext

    f32 = mybir.dt.float32
    f16 = mybir.dt.float16
    u8 = mybir.dt.uint8
    i32 = mybir.dt.int32
    AF = mybir.ActivationFunctionType
    ALU = mybir.AluOpType

    nc = bacc.Bacc("TRN2")
    xtf = nc.dram_tensor("xtf", [NIN, NB], f16, kind="ExternalInput")
    xtu = nc.dram_tensor("xtu", [NIN, NB], u8, kind="ExternalInput")
    # quad table: row = [pairA | pairB] = 2 x [f16 xF | u8 xU] = 12KB; one
    # descriptor delivers both operands of TWO neurons (slots 2j, 2j+1)
    xq = nc.dram_tensor("xq", [np_pairs, 6 * NB], u8, kind="ExternalInput")
    # same bytes viewed at half-row (pair) granularity, for slots 0-1
    xqh = nc.dram_tensor("xqh", [2 * np_pairs, 3 * NB], u8, kind="ExternalInput")
    NCALLS = KG // 2 + 1 + 2 * (SLOTS - KG)  # 8
    io = nc.dram_tensor("io", [128, NCALLS], i32, kind="ExternalInput")
    ac = nc.dram_tensor("ac", [128, 4, SLOTS], f32, kind="ExternalInput")
    yt = nc.dram_tensor("yt", [NN, NB], u8, kind="ExternalOutput")

    with TileContext(nc) as tc:
        with tc.tile_pool(name="all", bufs=1) as pool:
            it = pool.tile([128, NCALLS], i32)
            # load the offsets via the gpsimd (SWDGE) queue: it issues right
            # after the Q7 preamble, ~1.5us before the sync queue could
            nc.gpsimd.dma_start(it[:], io[:])
            act = pool.tile([128, 4, SLOTS], f32)
            nc.sync.dma_start(act[:], ac[:])

            gh = [pool.tile([128, 3 * NB], u8, name=f"gh{k}") for k in (0, 1)]
            gq = {j: pool.tile([128, 6 * NB], u8, name=f"gq{j}")
                  for j in (1, 2)}
            g1e6 = pool.tile([128, NB], f16)
            g1u7 = pool.tile([128, NB], u8)
            g2f = {s: pool.tile([128, NB], f16, name=f"g2f{s}") for s in (6, 7)}

            def gather(col, src, dst_ap):
                nc.gpsimd.indirect_dma_start(
                    out=dst_ap, out_offset=None, in_=src[:],
                    in_offset=bass.IndirectOffsetOnAxis(
                        ap=it[:, col:col + 1], axis=0),
                )

            # call order: half(slot0), half(slot1), alpha-g1-u8, quad(2-3),
            # alpha-g2-f16, quad(4-5), s6(g2-f16, g1-f16)
            order = [("h", 0), ("h", 1), ("u", (7, 1)), ("q", 1),
                     ("f", (7, 2)), ("q", 2), ("f", (6, 2)), ("f", (6, 1))]
            for col, (kind, what) in enumerate(order):
                if kind == "q":
                    gather(col, xq, gq[what][:])
                elif kind == "h":
                    gather(col, xqh, gh[what][:])
                else:
                    s, o = what
                    src = xtu if kind == "u" else xtf
                    if o == 1:
                        dst = g1u7[:] if s == 7 else g1e6[:]
                    else:
                        dst = g2f[s][:]
                    gather(col, src, dst)

            g1ap, g2ap = {}, {}
            for s in (0, 1):
                g1ap[s] = gh[s][:, 0:2 * NB].bitcast(f16)
                g2ap[s] = gh[s][:, 2 * NB:3 * NB]
            for s in (2, 3, 4, 5):
                j, k = s // 2, s % 2
                base = k * 3 * NB
                g1ap[s] = gq[j][:, base:base + 2 * NB].bitcast(f16)
                g2ap[s] = gq[j][:, base + 2 * NB:base + 3 * NB]
            g1ap[6], g2ap[6] = g1e6[:], g2f[6][:]
            g1ap[7], g2ap[7] = g1u7[:], g2f[7][:]

            t2 = [pool.tile([128, NB], f16, name=f"t2_{s}") for s in range(7)]
            t1 = [pool.tile([128, NB], f16, name=f"t1_{s}") for s in range(7)]
            qt = [pool.tile([128, NB], f16, name=f"q{s}") for s in range(7)]
            uv = pool.tile([128, 2, NB], f16)
            ot = [pool.tile([128, NB], u8, name=f"o{s}") for s in range(8)]

            def C(s, c):
                return act[:, c, s:s + 1]

            # ---- ACT order: t2_0, t2_1, alpha u/v, t2_2..t2_5 (t2_6 on DVE),
            # then u8-out finals for slots 0,1 (frees 2.6us of DVE)
            nc.scalar.activation(t2[0][:], g2ap[0], AF.Identity,
                                 bias=C(0, 1), scale=C(0, 0))
            nc.scalar.activation(t2[1][:], g2ap[1], AF.Identity,
                                 bias=C(1, 1), scale=C(1, 0))
            nc.scalar.activation(uv[:, 0, :], g1ap[7], AF.Identity,
                                 bias=C(7, 1), scale=C(7, 0))
            nc.scalar.activation(uv[:, 1, :], g1ap[7], AF.Identity,
                                 bias=C(7, 3), scale=C(7, 2))
            for s in (2, 3, 4, 5):
                nc.scalar.activation(t2[s][:], g2ap[s], AF.Identity,
                                     bias=C(s, 1), scale=C(s, 0))

            # ---- DVE stream, in arrival order
            def slotB(s, last=False, act_final=False):
                nc.vector.tensor_scalar_add(t1[s][:], g1ap[s], C(s, 2))
                nc.vector.tensor_mul(qt[s][:], t1[s][:], t2[s][:])
                if act_final:
                    # u8-out final on ACT: out = Identity(1.0*q + off)
                    nc.scalar.activation(ot[s][:], qt[s][:], AF.Identity,
                                         bias=C(s, 3), scale=1.0)
                else:
                    nc.vector.tensor_scalar_add(ot[s][:], qt[s][:], C(s, 3))
                if last:
                    h = NB // 2
                    nc.sync.dma_start(yt[s * 128:(s + 1) * 128, 0:h],
                                      ot[s][:, 0:h])
                    nc.scalar.dma_start(yt[s * 128:(s + 1) * 128, h:NB],
                                        ot[s][:, h:NB])
                else:
                    nc.sync.dma_start(yt[s * 128:(s + 1) * 128, :], ot[s][:])

            slotB(0, act_final=True)
            slotB(1, act_final=True)
            # alpha (slot 7): m = u*g2 ; out_u8 = m + v  (TT u8-out, 2x)
            nc.vector.tensor_mul(uv[:, 0, :], uv[:, 0, :], g2ap[7])
            nc.vector.tensor_add(ot[7][:], uv[:, 0, :], uv[:, 1, :])
            nc.sync.dma_start(yt[7 * 128:8 * 128, :], ot[7][:])
            slotB(2)
            slotB(3)
            slotB(4)
            slotB(5)
            # slot 6: t2 on DVE (f16 g2, 4x TS)
            nc.vector.tensor_scalar(t2[6][:], g2ap[6], C(6, 0), C(6, 1),
                                    ALU.mult, ALU.add)
            slotB(6, last=True)

    nc.compile()
    return nc


def _plan(w, conn_indices):
    ew = np.exp(w.astype(np.float64))
    probs = ew / ew.sum(axis=1, keepdims=True)
    A = probs @ GATE_COEF
    Apm = np.where(A[:, 3] == 0, 1e-12, A[:, 3])
    ratio = np.abs(A[:, 1] * A[:, 2]) / np.abs(Apm)
    order = np.argsort(ratio)
    n_alpha = NCORES * 128
    safe = order[:NNEUR - n_alpha]
    alpha = order[NNEUR - n_alpha:]

    a, b = conn_indices[:, 0].astype(int), conn_indices[:, 1].astype(int)
    from collections import defaultdict
    groups = defaultdict(list)
    for n in safe:
        n = int(n)
        if a[n] != b[n]:
            groups[(min(a[n], b[n]), max(a[n], b[n]))].append(n)
    deg = np.zeros(NIN, dtype=int)
    for (u, v), ns in groups.items():
        deg[u] += 1; deg[v] += 1
    used = np.zeros(NIN, dtype=int)
    CAP = 3  # covers 6268 safe neurons >= 6*1024 for KG=6
    matched_neurons = []
    pair_of = {}
    pairs = []
    for (u, v), ns in sorted(groups.items(),
                             key=lambda kv: min(deg[kv[0][0]], deg[kv[0][1]])):
        if used[u] < CAP and used[v] < CAP:
            used[u] += 1; used[v] += 1
            pid = len(pairs)
            pairs.append((u, v))
            for n in ns:
                pair_of[n] = pid
                matched_neurons.append(n)
    need_g = NCORES * 128 * KG
    assert len(matched_neurons) >= need_g, (len(matched_neurons), need_g)
    gamma = np.array(matched_neurons[:need_g])
    gset = set(gamma.tolist())
    rest = np.array([n for n in safe if int(n) not in gset])
    # trim pair table to used pairs only
    used_pids = sorted({pair_of[int(n)] for n in gamma})
    remap = {p: i for i, p in enumerate(used_pids)}
    pairs = [pairs[p] for p in used_pids]
    pair_of = {int(n): remap[pair_of[int(n)]] for n in gamma}
    return A, gamma, rest, alpha, pairs, pair_of


def _prep_core_inputs(x, w, conn_indices):
    xT = np.ascontiguousarray(x.T.astype(np.float64))
    xtf = xT.astype(np.float16)
    xtu = np.rint(xT * 255.0).astype(np.uint8)

    A, gamma, rest, alpha, pairs, pair_of = _plan(w, conn_indices)

    # pair rows (intermediate): [f16 xF | u8 xU]
    xp = np.empty((len(pairs), 3 * NB), dtype=np.uint8)
    pa = np.array([p[0] for p in pairs]); pb = np.array([p[1] for p in pairs])
    xp[:, 0:2 * NB] = xtf[pa].view(np.uint8)
    xp[:, 2 * NB:3 * NB] = xtu[pb]

    slot_map = np.empty((NCORES, SLOTS, 128), dtype=np.int64)
    slot_map[:, :KG] = gamma.reshape(NCORES, KG, 128)
    slot_map[:, KG:SLOTS - 1] = rest.reshape(NCORES, SLOTS - KG - 1, 128)
    slot_map[:, SLOTS - 1:] = alpha.reshape(NCORES, 1, 128)
    _CACHE["slot_map"] = slot_map

    # quads: one row per distinct (pairA, pairB) needed by some
    # (core, slot-pair, partition); delivers two slots in one descriptor
    quad_id = {}
    quad_cols = np.zeros((NCORES, KG // 2, 128), dtype=np.int32)
    for c in range(NCORES):
        for j in range(KG // 2):
            for p in range(128):
                key = (pair_of[int(slot_map[c, 2 * j, p])],
                       pair_of[int(slot_map[c, 2 * j + 1, p])])
                if key not in quad_id:
                    quad_id[key] = len(quad_id)
                quad_cols[c, j, p] = quad_id[key]
    nq = len(quad_id)
    xq = np.empty((nq, 6 * NB), dtype=np.uint8)
    for (qa, qb), qid in quad_id.items():
        xq[qid, 0:3 * NB] = xp[qa]
        xq[qid, 3 * NB:6 * NB] = xp[qb]
    _CACHE["np_pairs"] = nq

    A0, A1, A2, Ap = A[:, 0], A[:, 1], A[:, 2], A[:, 3]
    i1 = conn_indices[:, 0].astype(int)
    i2 = conn_indices[:, 1].astype(int)
    NCALLS = KG // 2 + 1 + 2 * (SLOTS - KG)

    maps = []
    for c in range(NCORES):
        io = np.zeros((128, NCALLS), dtype=np.int32)
        acx = np.zeros((128, 4, SLOTS), dtype=np.float32)
        for s in range(SLOTS):
            ns = slot_map[c, s]
            if s == 7:
                # alpha g1 is u8 now: a1 = g1u/255 folded into the scales
                acx[:, 0, s] = U8S * Ap[ns] / 255.0
                acx[:, 1, s] = U8S * A2[ns]
                acx[:, 2, s] = U8S * A1[ns] / 255.0
                acx[:, 3, s] = U8S * A0[ns] + U8B
                continue
            if s < KG:
                pid = np.array([pair_of[int(n)] for n in ns])
                pf = np.array([pairs[p][0] for p in pid])
                aF_is_i1 = (pf == i1[ns])
                A_F = np.where(aF_is_i1, A1[ns], A2[ns])
                A_U = np.where(aF_is_i1, A2[ns], A1[ns])
            else:
                A_F, A_U = A1[ns], A2[ns]
            K = A0[ns] - A_F * A_U / Ap[ns]
            acx[:, 1, s] = U8S * A_F
            acx[:, 2, s] = A_U / Ap[ns]
            acx[:, 3, s] = U8S * K + U8B
            if s == 6:
                acx[:, 0, s] = U8S * Ap[ns]           # f16 g2
            else:
                acx[:, 0, s] = U8S * Ap[ns] / 255.0   # u8 g2
        # io columns: must match _build_nc call order
        cols = [("h", 0), ("h", 1), ("a", (7, 1)), ("q", 1),
                ("a", (7, 2)), ("q", 2), ("a", (6, 2)), ("a", (6, 1))]
        for col, (kind, what) in enumerate(cols):
            if kind == "q":
                io[:, col] = quad_cols[c, what]
            elif kind == "h":
                io[:, col] = 2 * quad_cols[c, 0] + what
            else:
                s, o = what
                ns = slot_map[c, s]
                io[:, col] = i1[ns] if o == 1 else i2[ns]
        maps.append({
            "xtf": np.ascontiguousarray(xtf),
            "xtu": np.ascontiguousarray(xtu),
            "xq": xq,
            "xqh": xq.reshape(2 * nq, 3 * NB),
            "io": np.ascontiguousarray(io),
            "ac": np.ascontiguousarray(acx),
        })
    return maps


def run_cores(in_maps, trace=False):
    from concourse.bass_utils import run_bass_kernel_spmd

    if "nc" not in _CACHE:
        _CACHE["nc"] = _build_nc(_CACHE["np_pairs"])
    return run_bass_kernel_spmd(
        _CACHE["nc"], in_maps, core_ids=list(range(NCORES)), trace=trace
    )


def _assemble(results):
    slot_map = _CACHE["slot_map"]
    out = np.empty((BATCH, NNEUR), dtype=np.float32)
    for c in range(NCORES):
        y = (results[c]["yt"].astype(np.float32) - U8B) / U8S
        for s in range(SLOTS):
            out[:, slot_map[c, s]] = y[s * 128:(s + 1) * 128, :].T
    return out


def kernel(x, w, conn_indices):
    x = np.asarray(x, dtype=np.float32)
    w = np.asarray(w, dtype=np.float32)
    conn_indices = np.asarray(conn_indices)
    in_maps = _prep_core_inputs(x, w, conn_indices)
    res = run_cores(in_maps)
    return _assemble([r for r in res.results])


# revision 40
# speedup vs baseline: 1.0523x; 1.0523x over previous
"""LogicLayer Trainium2 kernel (v4: pair-gather + form-B, no fused STT).

out[b, n] = A0 + A1*a1 + A2*a2 + Ap*a1*a2  (A = softmax(w) @ C, host-precomputed)

Structure (per core: 1024 neurons x 2048 batch, 8 slots of 128 neurons):
  - Gathers via gpsimd.indirect_dma_start (~1.1 us issue per 128-row call).
    Descriptor count cut by PAIR-GATHERING: capacity-3 matching on the neuron
    graph (edges = (i1,i2)); matched operand pairs stored adjacent in a pair
    table so ONE descriptor fetches both operands: row = [f16 xF | u8 xU]
    (6KB). Slots 0-4 are paired (1 call each); slots 5-6 unpaired form-B
    (2 calls); slot 7 = form-A fallback for the worst-|A1*A2/Ap| neurons
    (factored form is numerically unsafe there). 11 calls/core.
  - form-B per slot (validated at rel_fro 2.3e-3):
      t2  = (253*Ap/255)*g2u8 + 253*A_F     ACT (u8 upconvert), slot6 on DVE
      t1  = g1f16 + A_U/Ap                  DVE tensor_scalar add (4x)
      q   = t1 * t2                         DVE tensor_tensor (2x), slot6 gpsimd
      u8  = q + (253*K + 1.5)               DVE tensor_scalar add, u8 out (2x)
  - form-A slot 7: u,v on ACT; m = u*g2 (TT); out = m + v written u8 directly
    (TT u8-out runs 2x, measured).
  - All outputs u8 = 253*out + 1.5 (HW rounds to nearest); host decodes.
  Measured op costs: ACT 2.08us; DVE TS f16 0.81, TS u8-out 1.28, TT 1.22;
  gpsimd TT ~4.4; NEVER u8-out on gpsimd (29us ucode path + DVE contention);
  scalar_tensor_tensor runs 1x (2.35us) - do not use.
"""

import numpy as np

BATCH = 2048
NIN = 8192
NNEUR = 8192
NCORES = 8
NN = NNEUR // NCORES
NB = BATCH
SLOTS = NN // 128
KG = 6                 # paired slots 0..5
U8S = 253.0
U8B = 1.5

GATE_COEF = np.array(
    [
        [0, 0, 0, 0], [0, 0, 0, 1], [0, 1, 0, -1], [0, 1, 0, 0],
        [0, 0, 1, -1], [0, 0, 1, 0], [0, 1, 1, -2], [0, 1, 1, -1],
        [1, -1, -1, 1], [1, -1, -1, 2], [1, 0, -1, 0], [1, 0, -1, 1],
        [1, -1, 0, 0], [1, -1, 0, 1], [1, 0, 0, -1], [1, 0, 0, 0],
    ],
    dtype=np.float64,
)

_CACHE = {}


def _build_nc(np_pairs):
    import concourse.bacc as bacc
    import concourse.bass as bass
    import concourse.mybir as mybir
    from concourse.tile import TileContext

    f32 = mybir.dt.float32
    f16 = mybir.dt.float16
    u8 = mybir.dt.uint8
    i32 = mybir.dt.int32
    AF = mybir.ActivationFunctionType
    ALU = mybir.AluOpType

    nc = bacc.Bacc("TRN2")
    xtf = nc.dram_tensor("xtf", [NIN, NB], f16, kind="ExternalInput")
    xtu = nc.dram_tensor("xtu", [NIN, NB], u8, kind="ExternalInput")
    # quad table: row = [pairA | pairB] = 2 x [f16 xF | u8 xU] = 12KB; one
    # descriptor delivers both operands of TWO neurons (slots 2j, 2j+1)
    xq = nc.dram_tensor("xq", [np_pairs, 6 * NB], u8, kind="ExternalInput")
    NCALLS = KG // 2 + 2 * (SLOTS - KG)  # 7
    io = nc.dram_tensor("io", [128, NCALLS], i32, kind="ExternalInput")
    ac = nc.dram_tensor("ac", [128, 4, SLOTS], f32, kind="ExternalInput")
    yt = nc.dram_tensor("yt", [NN, NB], u8, kind="ExternalOutput")

    with TileContext(nc) as tc:
        with tc.tile_pool(name="all", bufs=1) as pool:
            it = pool.tile([128, NCALLS], i32)
            # load the offsets via the gpsimd (SWDGE) queue: it issues right
            # after the Q7 preamble, ~1.5us before the sync queue could
            nc.gpsimd.dma_start(it[:], io[:])
            act = pool.tile([128, 4, SLOTS], f32)
            nc.sync.dma_start(act[:], ac[:])

            gq = [pool.tile([128, 6 * NB], u8, name=f"gq{j}")
                  for j in range(KG // 2)]
            g1e6 = pool.tile([128, NB], f16)
            g1u7 = pool.tile([128, NB], u8)
            g2f = {s: pool.tile([128, NB], f16, name=f"g2f{s}") for s in (6, 7)}

            def gather(col, src, dst_ap):
                nc.gpsimd.indirect_dma_start(
                    out=dst_ap, out_offset=None, in_=src[:],
                    in_offset=bass.IndirectOffsetOnAxis(
                        ap=it[:, col:col + 1], axis=0),
                )

            # call order: quad(slots 0-1), quad(2-3), alpha(g1-u8, g2-f16),
            # quad(4-5), s6(g2-f16, g1-f16)
            order = [("q", 0), ("q", 1), ("u", (7, 1)), ("f", (7, 2)),
                     ("q", 2), ("f", (6, 2)), ("f", (6, 1))]
            for col, (kind, what) in enumerate(order):
                if kind == "q":
                    gather(col, xq, gq[what][:])
                else:
                    s, o = what
                    src = xtu if kind == "u" else xtf
                    if o == 1:
                        dst = g1u7[:] if s == 7 else g1e6[:]
                    else:
                        dst = g2f[s][:]
                    gather(col, src, dst)

            g1ap, g2ap = {}, {}
            for s in range(KG):
                j, k = s // 2, s % 2
                base = k * 3 * NB
                g1ap[s] = gq[j][:, base:base + 2 * NB].bitcast(f16)
                g2ap[s] = gq[j][:, base + 2 * NB:base + 3 * NB]
            g1ap[6], g2ap[6] = g1e6[:], g2f[6][:]
            g1ap[7], g2ap[7] = g1u7[:], g2f[7][:]

            t2 = [pool.tile([128, NB], f16, name=f"t2_{s}") for s in range(7)]
            t1 = [pool.tile([128, NB], f16, name=f"t1_{s}") for s in range(7)]
            qt = [pool.tile([128, NB], f16, name=f"q{s}") for s in range(7)]
            uv = pool.tile([128, 2, NB], f16)
            ot = [pool.tile([128, NB], u8, name=f"o{s}") for s in range(8)]

            def C(s, c):
                return act[:, c, s:s + 1]

            # ---- ACT order: t2_0, t2_1, alpha u/v, t2_2..t2_5 (t2_6 on DVE),
            # then u8-out finals for slots 0,1 (frees 2.6us of DVE)
            nc.scalar.activation(t2[0][:], g2ap[0], AF.Identity,
                                 bias=C(0, 1), scale=C(0, 0))
            nc.scalar.activation(t2[1][:], g2ap[1], AF.Identity,
                                 bias=C(1, 1), scale=C(1, 0))
            nc.scalar.activation(uv[:, 0, :], g1ap[7], AF.Identity,
                                 bias=C(7, 1), scale=C(7, 0))
            nc.scalar.activation(uv[:, 1, :], g1ap[7], AF.Identity,
                                 bias=C(7, 3), scale=C(7, 2))
            for s in (2, 3, 4, 5):
                nc.scalar.activation(t2[s][:], g2ap[s], AF.Identity,
                                     bias=C(s, 1), scale=C(s, 0))

            # ---- DVE stream, in arrival order
            def slotB(s, last=False, act_final=False):
                nc.vector.tensor_scalar_add(t1[s][:], g1ap[s], C(s, 2))
                nc.vector.tensor_mul(qt[s][:], t1[s][:], t2[s][:])
                if act_final:
                    # u8-out final on ACT: out = Identity(1.0*q + off)
                    nc.scalar.activation(ot[s][:], qt[s][:], AF.Identity,
                                         bias=C(s, 3), scale=1.0)
                else:
                    nc.vector.tensor_scalar_add(ot[s][:], qt[s][:], C(s, 3))
                if last:
                    h = NB // 2
                    nc.sync.dma_start(yt[s * 128:(s + 1) * 128, 0:h],
                                      ot[s][:, 0:h])
                    nc.scalar.dma_start(yt[s * 128:(s + 1) * 128, h:NB],
                                        ot[s][:, h:NB])
                else:
                    nc.sync.dma_start(yt[s * 128:(s + 1) * 128, :], ot[s][:])

            slotB(0, act_final=True)
            slotB(1, act_final=True)
            # alpha (slot 7): m = u*g2 ; out_u8 = m + v  (TT u8-out, 2x)
            nc.vector.tensor_mul(uv[:, 0, :], uv[:, 0, :], g2ap[7])
            nc.vector.tensor_add(ot[7][:], uv[:, 0, :], uv[:, 1, :])
            nc.sync.dma_start(yt[7 * 128:8 * 128, :], ot[7][:])
            slotB(2)
            slotB(3)
            slotB(4)
            slotB(5)
            # slot 6: t2 on DVE (f16 g2, 4x TS)
            nc.vector.tensor_scalar(t2[6][:], g2ap[6], C(6, 0), C(6, 1),
                                    ALU.mult, ALU.add)
            slotB(6, last=True)

    nc.compile()
    return nc


def _plan(w, conn_indices):
    ew = np.exp(w.astype(np.float64))
    probs = ew / ew.sum(axis=1, keepdims=True)
    A = probs @ GATE_COEF
    Apm = np.where(A[:, 3] == 0, 1e-12, A[:, 3])
    ratio = np.abs(A[:, 1] * A[:, 2]) / np.abs(Apm)
    order = np.argsort(ratio)
    n_alpha = NCORES * 128
    safe = order[:NNEUR - n_alpha]
    alpha = order[NNEUR - n_alpha:]

    a, b = conn_indices[:, 0].astype(int), conn_indices[:, 1].astype(int)
    from collections import defaultdict
    groups = defaultdict(list)
    for n in safe:
        n = int(n)
        if a[n] != b[n]:
            groups[(min(a[n], b[n]), max(a[n], b[n]))].append(n)
    deg = np.zeros(NIN, dtype=int)
    for (u, v), ns in groups.items():
        deg[u] += 1; deg[v] += 1
    used = np.zeros(NIN, dtype=int)
    CAP = 3  # covers 6268 safe neurons >= 6*1024 for KG=6
    matched_neurons = []
    pair_of = {}
    pairs = []
    for (u, v), ns in sorted(groups.items(),
                             key=lambda kv: min(deg[kv[0][0]], deg[kv[0][1]])):
        if used[u] < CAP and used[v] < CAP:
            used[u] += 1; used[v] += 1
            pid = len(pairs)
            pairs.append((u, v))
            for n in ns:
                pair_of[n] = pid
                matched_neurons.append(n)
    need_g = NCORES * 128 * KG
    assert len(matched_neurons) >= need_g, (len(matched_neurons), need_g)
    gamma = np.array(matched_neurons[:need_g])
    gset = set(gamma.tolist())
    rest = np.array([n for n in safe if int(n) not in gset])
    # trim pair table to used pairs only
    used_pids = sorted({pair_of[int(n)] for n in gamma})
    remap = {p: i for i, p in enumerate(used_pids)}
    pairs = [pairs[p] for p in used_pids]
    pair_of = {int(n): remap[pair_of[int(n)]] for n in gamma}
    return A, gamma, rest, alpha, pairs, pair_of


def _prep_core_inputs(x, w, conn_indices):
    xT = np.ascontiguousarray(x.T.astype(np.float64))
    xtf = xT.astype(np.float16)
    xtu = np.rint(xT * 255.0).astype(np.uint8)

    A, gamma, rest, alpha, pairs, pair_of = _plan(w, conn_indices)

    # pair rows (intermediate): [f16 xF | u8 xU]
    xp = np.empty((len(pairs), 3 * NB), dtype=np.uint8)
    pa = np.array([p[0] for p in pairs]); pb = np.array([p[1] for p in pairs])
    xp[:, 0:2 * NB] = xtf[pa].view(np.uint8)
    xp[:, 2 * NB:3 * NB] = xtu[pb]

    slot_map = np.empty((NCORES, SLOTS, 128), dtype=np.int64)
    slot_map[:, :KG] = gamma.reshape(NCORES, KG, 128)
    slot_map[:, KG:SLOTS - 1] = rest.reshape(NCORES, SLOTS - KG - 1, 128)
    slot_map[:, SLOTS - 1:] = alpha.reshape(NCORES, 1, 128)
    _CACHE["slot_map"] = slot_map

    # quads: one row per distinct (pairA, pairB) needed by some
    # (core, slot-pair, partition); delivers two slots in one descriptor
    quad_id = {}
    quad_cols = np.zeros((NCORES, KG // 2, 128), dtype=np.int32)
    for c in range(NCORES):
        for j in range(KG // 2):
            for p in range(128):
                key = (pair_of[int(slot_map[c, 2 * j, p])],
                       pair_of[int(slot_map[c, 2 * j + 1, p])])
                if key not in quad_id:
                    quad_id[key] = len(quad_id)
                quad_cols[c, j, p] = quad_id[key]
    nq = len(quad_id)
    xq = np.empty((nq, 6 * NB), dtype=np.uint8)
    for (qa, qb), qid in quad_id.items():
        xq[qid, 0:3 * NB] = xp[qa]
        xq[qid, 3 * NB:6 * NB] = xp[qb]
    _CACHE["np_pairs"] = nq

    A0, A1, A2, Ap = A[:, 0], A[:, 1], A[:, 2], A[:, 3]
    i1 = conn_indices[:, 0].astype(int)
    i2 = conn_indices[:, 1].astype(int)
    NCALLS = KG // 2 + 2 * (SLOTS - KG)

    maps = []
    for c in range(NCORES):
        io = np.zeros((128, NCALLS), dtype=np.int32)
        acx = np.zeros((128, 4, SLOTS), dtype=np.float32)
        for s in range(SLOTS):
            ns = slot_map[c, s]
            if s == 7:
                # alpha g1 is u8 now: a1 = g1u/255 folded into the scales
                acx[:, 0, s] = U8S * Ap[ns] / 255.0
                acx[:, 1, s] = U8S * A2[ns]
                acx[:, 2, s] = U8S * A1[ns] / 255.0
                acx[:, 3, s] = U8S * A0[ns] + U8B
                continue
            if s < KG:
                pid = np.array([pair_of[int(n)] for n in ns])
                pf = np.array([pairs[p][0] for p in pid])
                aF_is_i1 = (pf == i1[ns])
                A_F = np.where(aF_is_i1, A1[ns], A2[ns])
                A_U = np.where(aF_is_i1, A2[ns], A1[ns])
            else:
                A_F, A_U = A1[ns], A2[ns]
            K = A0[ns] - A_F * A_U / Ap[ns]
            acx[:, 1, s] = U8S * A_F
            acx[:, 2, s] = A_U / Ap[ns]
            acx[:, 3, s] = U8S * K + U8B
            if s == 6:
                acx[:, 0, s] = U8S * Ap[ns]           # f16 g2
            else:
                acx[:, 0, s] = U8S * Ap[ns] / 255.0   # u8 g2
        # io columns: must match _build_nc call order
        cols = [("q", 0), ("q", 1), ("a", (7, 1)), ("a", (7, 2)),
                ("q", 2), ("a", (6, 2)), ("a", (6, 1))]
        for col, (kind, what) in enumerate(cols):
            if kind == "q":
                io[:, col] = quad_cols[c, what]
            else:
                s, o = what
                ns = slot_map[c, s]
                io[:, col] = i1[ns] if o == 1 else i2[ns]
        maps.append({
            "xtf": np.ascontiguousarray(xtf),
            "xtu": np.ascontiguousarray(xtu),
            "xq": xq,
            "io": np.ascontiguousarray(io),
            "ac": np.ascontiguousarray(acx),
        })
    return maps


def run_cores(in_maps, trace=False):
    from concourse.bass_utils import run_bass_kernel_spmd

    if "nc" not in _CACHE:
        _CACHE["nc"] = _build_nc(_CACHE["np_pairs"])
    return run_bass_kernel_spmd(
        _CACHE["nc"], in_maps, core_ids=list(range(NCORES)), trace=trace
    )


def _assemble(results):
    slot_map = _CACHE["slot_map"]
    out = np.empty((BATCH, NNEUR), dtype=np.float32)
    for c in range(NCORES):
        y = (results[c]["yt"].astype(np.float32) - U8B) / U8S
        for s in range(SLOTS):
            out[:, slot_map[c, s]] = y[s * 128:(s + 1) * 128, :].T
    return out


def kernel(x, w, conn_indices):
    x = np.asarray(x, dtype=np.float32)
    w = np.asarray(w, dtype=np.float32)
    conn_indices = np.asarray(conn_indices)
    in_maps = _prep_core_inputs(x, w, conn_indices)
    res = run_cores(in_maps)
    return _assemble([r for r in res.results])
